# revision 10
# baseline (speedup 1.0000x reference)
"""AdditiveAttention Bass kernel for 8 TRN2 NeuronCores (data-parallel over batch).

e = w^T tanh(W h + U V^T + b); beta = softmax(e); out = sum_s beta_s V_s

Sharding: batch dim (32) split 4-per-core; weights replicated.
Compute in bf16 on TensorE with f32 PSUM accumulation.
"""
import numpy as np
from contextlib import ExitStack

import concourse.bass as bass
import concourse.tile as tile
from concourse import bacc, mybir
from concourse.bass_utils import run_bass_kernel_spmd
from concourse.masks import make_identity

N_CORES = 8
B, S, VLEN, HID, HLEN = 32, 2048, 1024, 1024, 1024
B_LOC = B // N_CORES          # 4 batches per core
NT = S // 128                 # 16 s-blocks of 128
NC_V = VLEN // 128            # 8 v-chunks
NC_D = HID // 128             # 8 d-chunks
NC_L = HLEN // 128            # 8 l-chunks
NST = S // 512                # 4 s-tiles of 512
F32 = mybir.dt.float32
BF16 = mybir.dt.bfloat16
AF = mybir.ActivationFunctionType


def build_kernel(debug=False):
    nc = bacc.Bacc("TRN2", target_bir_lowering=False, debug=False,
                   num_devices=N_CORES)
    h_ext = nc.declare_dram_parameter("h", [B_LOC, HLEN], F32, isOutput=False)
    V_ext = nc.declare_dram_parameter("V", [B_LOC, S, VLEN], F32, isOutput=False)
    Ww_ext = nc.declare_dram_parameter("W_w", [HID, HLEN], F32, isOutput=False)
    Wb_ext = nc.declare_dram_parameter("W_b", [HID], F32, isOutput=False)
    Uw_ext = nc.declare_dram_parameter("U_w", [HID, VLEN], F32, isOutput=False)
    ww_ext = nc.declare_dram_parameter("w_w", [HID], F32, isOutput=False)
    out_ext = nc.declare_dram_parameter("out", [B_LOC, VLEN], F32, isOutput=True)
    if debug:
        dbg_hp_ext = nc.declare_dram_parameter(
            "dbg_hpT", [128, NC_D, B_LOC], F32, isOutput=True)
        dbg_e_ext = nc.declare_dram_parameter(
            "dbg_e", [B_LOC, NST, 512], F32, isOutput=True)
        dbg_vt_ext = nc.declare_dram_parameter(
            "dbg_vt0", [128, NC_V, S], F32, isOutput=True)
        dbg_vp_ext = nc.declare_dram_parameter(
            "dbg_vp0", [128, NC_D, S], F32, isOutput=True)
        dbg_th_ext = nc.declare_dram_parameter(
            "dbg_th0", [128, NC_D, S], F32, isOutput=True)

    with tile.TileContext(nc) as tc, ExitStack() as ctx:
        const_pool = ctx.enter_context(tc.tile_pool(name="const", bufs=1))
        wt_pool = ctx.enter_context(tc.tile_pool(name="wt", bufs=1))
        dram_pool = ctx.enter_context(tc.tile_pool(name="dram", bufs=1, space="DRAM"))
        nat_pool = ctx.enter_context(tc.tile_pool(name="nat", bufs=2))
        vt_pool = ctx.enter_context(tc.tile_pool(name="vt", bufs=2))
        tanh_pool = ctx.enter_context(tc.tile_pool(name="tanh", bufs=4))
        p_pool = ctx.enter_context(tc.tile_pool(name="p", bufs=2))
        o_pool = ctx.enter_context(tc.tile_pool(name="o", bufs=2))
        if debug:
            dbg_pool = ctx.enter_context(tc.tile_pool(name="dbg", bufs=1))
        ps_tr_pool = ctx.enter_context(tc.tile_pool(name="ps_tr", bufs=2, space="PSUM"))
        ps_vp_pool = ctx.enter_context(tc.tile_pool(name="ps_vp", bufs=2, space="PSUM"))
        ps_e_pool = ctx.enter_context(tc.tile_pool(name="ps_e", bufs=2, space="PSUM"))
        ps_pt_pool = ctx.enter_context(tc.tile_pool(name="ps_pt", bufs=1, space="PSUM"))
        ps_o_pool = ctx.enter_context(tc.tile_pool(name="ps_o", bufs=1, space="PSUM"))

        # ---- constants ----
        id_bf = const_pool.tile([128, 128], BF16)
        make_identity(nc, id_bf[:])
        id_f32 = const_pool.tile([128, 128], F32)
        make_identity(nc, id_f32[:])

        # ---- weight prep: bf16 staging in DRAM, then xbar-transpose in ----
        Uw_bf = dram_pool.tile([HID, VLEN], BF16)
        nc.gpsimd.dma_start(Uw_bf[:], Uw_ext[:])        # f32 -> bf16 cast
        Ww_bf = dram_pool.tile([HID, HLEN], BF16)
        nc.gpsimd.dma_start(Ww_bf[:], Ww_ext[:])

        # UwT[v_part, c, d] : U_w^T chunks (v on partitions)
        UwT = wt_pool.tile([128, NC_V, HID], BF16)
        # WwT[l_part, c, d] : W_w^T chunks (l on partitions)
        WwT = wt_pool.tile([128, NC_L, HID], BF16)
        for c in range(NC_V):
            nc.sync.dma_start(UwT[:, c, :], Uw_bf[:, c * 128:(c + 1) * 128],
                              transpose=True)
        for c in range(NC_L):
            nc.sync.dma_start(WwT[:, c, :], Ww_bf[:, c * 128:(c + 1) * 128],
                              transpose=True)

        # ---- hstack: rows 0-3 = h, row 4 = W_b, row 5 = w_w ----
        hstack = wt_pool.tile([6, HLEN], F32)
        nc.sync.dma_start(hstack[0:B_LOC, :], h_ext[:, :])
        nc.sync.dma_start(hstack[4:5, :],
                          Wb_ext[:].rearrange("(a v) -> a v", a=1))
        nc.sync.dma_start(hstack[5:6, :],
                          ww_ext[:].rearrange("(a v) -> a v", a=1))

        hT = wt_pool.tile([128, NC_L, B_LOC], BF16)   # h^T chunks (l on part)
        WbT = wt_pool.tile([128, NC_D], F32)          # W_b^T chunks (d on part)
        wwT = wt_pool.tile([128, NC_D], BF16)         # w_w^T chunks (d on part)
        for c in range(NC_L):
            hsT = ps_vp_pool.tile([128, 6], F32, tag="vp")
            nc.tensor.transpose(hsT[:], hstack[:, c * 128:(c + 1) * 128],
                                id_f32[0:6, 0:6])
            nc.vector.tensor_copy(hT[:, c, :], hsT[:, 0:B_LOC])
            nc.vector.tensor_copy(WbT[:, c:c + 1], hsT[:, 4:5])
            nc.vector.tensor_copy(wwT[:, c:c + 1], hsT[:, 5:6])

        # ---- h_projT[d, b] = W_w h + W_b (transposed) ----
        hpT = wt_pool.tile([128, NC_D, B_LOC], F32)
        for dc in range(NC_D):
            ph = ps_vp_pool.tile([128, B_LOC], F32, tag="vp")
            for lc in range(NC_L):
                nc.tensor.matmul(ph[:], WwT[:, lc, dc * 128:(dc + 1) * 128],
                                 hT[:, lc, :],
                                 start=(lc == 0), stop=(lc == NC_L - 1))
            nc.vector.tensor_scalar_add(hpT[:, dc, :], ph[:], WbT[:, dc:dc + 1])
        if debug:
            nc.sync.dma_start(dbg_hp_ext[:], hpT[:])

        # ---- main per-batch loop ----
        for b in range(B_LOC):
            # V natural layout, cast to bf16 during DMA: nat[s_part, t, v]
            nat = nat_pool.tile([128, NT, VLEN], BF16, tag="nat")
            for t in range(NT):
                nc.gpsimd.dma_start(nat[:, t, :],
                                    V_ext[b, t * 128:(t + 1) * 128, :])

            # V^T via PE transpose: vt[v_part, c, s]
            vt = vt_pool.tile([128, NC_V, S], BF16, tag="vt")
            for c in range(NC_V):
                for q in range(NT // 4):
                    tr = ps_tr_pool.tile([128, 512], BF16, tag="tr")
                    for j in range(4):
                        t = 4 * q + j
                        nc.tensor.transpose(tr[:, j * 128:(j + 1) * 128],
                                            nat[:, t, c * 128:(c + 1) * 128],
                                            id_bf[:, :])
                    nc.vector.tensor_copy(vt[:, c, q * 512:(q + 1) * 512], tr[:])

            # pass 1: e[s] = sum_d w_d tanh(h_proj_d + (U V^T)[d, s])
            # e accumulated as natural rows [1, 512] - one PSUM accumulation
            # group per s-tile (start=True clears has_written for the WHOLE
            # bank, so groups must not interleave within a bank).
            zparts = o_pool.tile([1, NST], F32, tag="zp")
            p_nat = p_pool.tile([1, S], BF16, tag="pnat")
            for st in range(NST):
                er = ps_e_pool.tile([1, 512], F32, tag="er")
                for dc in range(NC_D):
                    vp = ps_vp_pool.tile([128, 512], F32, tag="vp")
                    for vc in range(NC_V):
                        nc.tensor.matmul(
                            vp[:], UwT[:, vc, dc * 128:(dc + 1) * 128],
                            vt[:, vc, st * 512:(st + 1) * 512],
                            start=(vc == 0), stop=(vc == NC_V - 1))
                    if debug and b == 0:
                        vpdbg = dbg_pool.tile([128, 512], F32, tag="vpdbg")
                        nc.vector.tensor_copy(vpdbg[:], vp[:])
                        nc.sync.dma_start(
                            dbg_vp_ext[:, dc, st * 512:(st + 1) * 512],
                            vpdbg[:])
                    th = tanh_pool.tile([128, 512], BF16, tag="th")
                    nc.scalar.activation(th[:], vp[:], AF.Tanh,
                                         bias=hpT[:, dc, b:b + 1])
                    if debug and b == 0:
                        thdbg = dbg_pool.tile([128, 512], F32, tag="thdbg")
                        nc.vector.tensor_copy(thdbg[:], th[:])
                        nc.sync.dma_start(
                            dbg_th_ext[:, dc, st * 512:(st + 1) * 512],
                            thdbg[:])
                    nc.tensor.matmul(er[:], wwT[:, dc:dc + 1], th[:],
                                     start=(dc == 0), stop=(dc == NC_D - 1))
                if debug:
                    erdbg = dbg_pool.tile([1, 512], F32, tag="erdbg")
                    nc.vector.tensor_copy(erdbg[:], er[:])
                    nc.sync.dma_start(dbg_e_ext[b, st, :],
                                      erdbg[:].rearrange("a v -> (a v)"))
                # softmax numerator (e bounded, skip max-subtraction);
                # accum_out gives the partial sum of exp for Z
                nc.scalar.activation(p_nat[0:1, st * 512:(st + 1) * 512],
                                     er[:], AF.Exp,
                                     accum_out=zparts[0:1, st:st + 1])

            if debug:
                if b == 0:
                    for c in range(NC_V):
                        vdbg = dbg_pool.tile([128, S], F32, tag="vdbg")
                        nc.vector.tensor_copy(vdbg[:], vt[:, c, :])
                        nc.sync.dma_start(dbg_vt_ext[:, c, :], vdbg[:])

            # transpose p rows back to columns for pass-2 lhsT
            # (pad columns to 4B: PSUM accesses must be 4-byte aligned)
            ps_pt = ps_pt_pool.tile([128, NT, 2], BF16, tag="pt")
            for t in range(NT):
                nc.tensor.transpose(ps_pt[:, t, 0:1],
                                    p_nat[0:1, t * 128:(t + 1) * 128],
                                    id_bf[0:1, 0:1])
            pT = p_pool.tile([128, NT], BF16, tag="pT")
            nc.vector.tensor_copy(pT[:], ps_pt[:, :, 0])

            # pass 2: out_un[v] = sum_s p_s V[s, v]
            out_un = o_pool.tile([1, VLEN], F32, tag="oun")
            for half in range(2):
                po = ps_o_pool.tile([1, 512], F32, tag="po")
                for t in range(NT):
                    nc.tensor.matmul(po[:], pT[:, t:t + 1],
                                     nat[:, t, half * 512:(half + 1) * 512],
                                     start=(t == 0), stop=(t == NT - 1))
                nc.vector.tensor_copy(out_un[0:1, half * 512:(half + 1) * 512],
                                      po[:])
            zsum = o_pool.tile([1, 1], F32, tag="zs")
            nc.vector.reduce_sum(zsum[:], zparts[:], axis=mybir.AxisListType.X)
            zinv = o_pool.tile([1, 1], F32, tag="zi")
            nc.vector.reciprocal(zinv[:], zsum[:])
            out_fin = o_pool.tile([1, VLEN], F32, tag="ofin")
            nc.vector.tensor_scalar_mul(out_fin[:], out_un[:], zinv[:])
            nc.sync.dma_start(out_ext[b:b + 1, :], out_fin[:])

    nc.compile()
    return nc


_cached_nc = None


def _get_nc():
    global _cached_nc
    if _cached_nc is None:
        _cached_nc = build_kernel()
    return _cached_nc


def make_in_maps(h, V, W_w, W_b, U_w, w_w):
    h = np.ascontiguousarray(np.asarray(h, dtype=np.float32))
    V = np.ascontiguousarray(np.asarray(V, dtype=np.float32))
    W_w = np.ascontiguousarray(np.asarray(W_w, dtype=np.float32))
    W_b = np.ascontiguousarray(np.asarray(W_b, dtype=np.float32))
    U_w = np.ascontiguousarray(np.asarray(U_w, dtype=np.float32))
    w_w = np.ascontiguousarray(np.asarray(w_w, dtype=np.float32))
    in_maps = []
    for i in range(N_CORES):
        sl = slice(i * B_LOC, (i + 1) * B_LOC)
        in_maps.append({
            "h": h[sl], "V": V[sl],
            "W_w": W_w, "W_b": W_b, "U_w": U_w, "w_w": w_w,
        })
    return in_maps


def kernel(h, V, W_w, W_b, U_w, w_w):
    nc = _get_nc()
    in_maps = make_in_maps(h, V, W_w, W_b, U_w, w_w)
    res = run_bass_kernel_spmd(nc, in_maps, list(range(N_CORES)))
    out = np.concatenate([res.results[i]["out"] for i in range(N_CORES)],
                         axis=0)
    return np.asarray(out, dtype=np.float32)


if __name__ == "__main__":
    import jax
    rng = np.random.default_rng(0)
    # quick self-exercise with random data (not the reference check)
    h = rng.standard_normal((B, HLEN), dtype=np.float32)
    V = rng.standard_normal((B, S, VLEN), dtype=np.float32)
    W_w = rng.standard_normal((HID, HLEN), dtype=np.float32) / np.sqrt(HLEN)
    W_b = rng.standard_normal((HID,), dtype=np.float32) / np.sqrt(HLEN)
    U_w = rng.standard_normal((HID, VLEN), dtype=np.float32) / np.sqrt(VLEN)
    w_w = rng.standard_normal((HID,), dtype=np.float32) / np.sqrt(HID)
    out = kernel(h, V, W_w, W_b, U_w, w_w)
    print("out", out.shape, out.dtype, float(np.abs(out).mean()))


# revision 22
# speedup vs baseline: 171.9925x; 171.9925x over previous
"""AdditiveAttention Bass kernel for 8 TRN2 NeuronCores (data-parallel over batch).

e = w^T tanh(W h + U V^T + b); beta = softmax(e); out = sum_s beta_s V_s

Sharding: batch dim (32) split 4-per-core; weights replicated.
Compute in bf16 on TensorE with f32 PSUM accumulation.

Key structure per core (4 local batches):
 - V cast-loaded f32->bf16 by SWDGE DMA into natural layout nat[s,t,v]
 - V^T produced on-chip by PE transpose (DMA-xbar transpose serializes
   against concurrent DMAs, so it is avoided entirely)
 - pass 1 computes V_projT[d,s] tiles = UwT.T @ V^T with d on partitions,
   so h_proj enters as the ScalarE activation per-partition bias and
   e[s] = w^T tanh(.) is a cheap [1,512] PSUM-accumulated matmul row
 - softmax without max-subtraction (e is O(1) bounded); Z via accum_out
 - pass 2: out = sum_s p_s V_s with p columns as the stationary operand
   against natural-layout V tiles; normalize by 1/Z at the end

PSUM hazard note: start=True clears has_written bits for the WHOLE bank,
so accumulation groups never interleave within a bank.
"""
import numpy as np
from contextlib import ExitStack

import concourse.bass as bass
import concourse.tile as tile
from concourse import bacc, mybir
from concourse.bass_utils import run_bass_kernel_spmd
from concourse.masks import make_identity

N_CORES = 8
B, S, VLEN, HID, HLEN = 32, 2048, 1024, 1024, 1024
B_LOC = B // N_CORES          # 4 batches per core
NT = S // 128                 # 16 s-blocks of 128
NC_V = VLEN // 128            # 8 v-chunks
NC_D = HID // 128             # 8 d-chunks
NC_L = HLEN // 128            # 8 l-chunks
NST = S // 512                # 4 s-tiles of 512
F32 = mybir.dt.float32
BF16 = mybir.dt.bfloat16
AF = mybir.ActivationFunctionType

DEFAULT_CFG = dict(nat_bufs=2, vt_bufs=2, tanh_bufs=4, vp_bufs=3, tr_bufs=2,
                   er_bufs=2, batch_dma=False, st_pair=False, pipelined=False,
                   xbar_vt=False, merge_popt=True, mm_tr=True, alt_copy=True)


def build_kernel(repeat=1, ablate=(), cfg=None, debug=False):
    _cfg = dict(DEFAULT_CFG)
    _cfg.update(cfg or {})
    cfg = _cfg

    nc = bacc.Bacc("TRN2", target_bir_lowering=False, debug=False,
                   num_devices=N_CORES)
    h_ext = nc.declare_dram_parameter("h", [B_LOC, HLEN], F32, isOutput=False)
    V_ext = nc.declare_dram_parameter("V", [B_LOC, S, VLEN], F32, isOutput=False)
    Ww_ext = nc.declare_dram_parameter("W_w", [HID, HLEN], F32, isOutput=False)
    Wb_ext = nc.declare_dram_parameter("W_b", [HID], F32, isOutput=False)
    Uw_ext = nc.declare_dram_parameter("U_w", [HID, VLEN], F32, isOutput=False)
    ww_ext = nc.declare_dram_parameter("w_w", [HID], F32, isOutput=False)
    out_ext = nc.declare_dram_parameter("out", [B_LOC, VLEN], F32, isOutput=True)

    with tile.TileContext(nc) as tc, ExitStack() as ctx:
        const_pool = ctx.enter_context(tc.tile_pool(name="const", bufs=1))
        wt_pool = ctx.enter_context(tc.tile_pool(name="wt", bufs=1))
        nat_pool = ctx.enter_context(
            tc.tile_pool(name="nat", bufs=cfg["nat_bufs"]))
        vt_pool = ctx.enter_context(
            tc.tile_pool(name="vt", bufs=cfg["vt_bufs"]))
        tanh_pool = ctx.enter_context(
            tc.tile_pool(name="tanh", bufs=cfg["tanh_bufs"]))
        p_pool = ctx.enter_context(tc.tile_pool(name="p", bufs=2))
        o_pool = ctx.enter_context(tc.tile_pool(name="o", bufs=2))
        ps_tr_pool = ctx.enter_context(
            tc.tile_pool(name="ps_tr", bufs=cfg["tr_bufs"], space="PSUM"))
        ps_vp_pool = ctx.enter_context(
            tc.tile_pool(name="ps_vp", bufs=cfg["vp_bufs"], space="PSUM"))
        ps_e_pool = ctx.enter_context(
            tc.tile_pool(name="ps_e", bufs=cfg["er_bufs"], space="PSUM"))
        ps_pt_pool = ctx.enter_context(
            tc.tile_pool(name="ps_pt", bufs=1, space="PSUM"))
        ps_o_pool = ctx.enter_context(
            tc.tile_pool(name="ps_o", bufs=1, space="PSUM"))

        # ---- constants ----
        id_bf = const_pool.tile([128, 128], BF16)
        make_identity(nc, id_bf[:])
        id_f32 = const_pool.tile([128, 128], F32)
        make_identity(nc, id_f32[:])

        # ---- weight prep: cast-load natural, then transpose on PE ----
        # UwT[v_part, c, d] : U_w^T chunks (v on partitions)
        UwT = wt_pool.tile([128, NC_V, HID], BF16)
        # WwT[l_part, c, d] : W_w^T chunks (l on partitions)
        WwT = wt_pool.tile([128, NC_L, HID], BF16)
        for (w_ext, wT) in ((Uw_ext, UwT), (Ww_ext, WwT)):
            wnat = nat_pool.tile([128, NC_D, VLEN], BF16, tag="nat")
            for dc in range(NC_D):
                nc.gpsimd.dma_start(wnat[:, dc, :],
                                    w_ext[dc * 128:(dc + 1) * 128, :])
            for vc in range(NC_V):
                for dq in range(NC_D // 4):
                    if cfg["mm_tr"]:
                        trwf = ps_tr_pool.tile([128, 512], F32, tag="tr")
                        for j in range(4):
                            dc = 4 * dq + j
                            nc.tensor.matmul(
                                trwf[:, j * 128:(j + 1) * 128],
                                wnat[:, dc, vc * 128:(vc + 1) * 128],
                                id_bf[:, :], start=True, stop=True)
                        nc.vector.tensor_copy(
                            wT[:, vc, dq * 512:(dq + 1) * 512], trwf[:])
                        continue
                    trw = ps_tr_pool.tile([128, 512], BF16, tag="tr")
                    for j in range(4):
                        dc = 4 * dq + j
                        nc.tensor.transpose(
                            trw[:, j * 128:(j + 1) * 128],
                            wnat[:, dc, vc * 128:(vc + 1) * 128],
                            id_bf[:, :])
                    nc.vector.tensor_copy(
                        wT[:, vc, dq * 512:(dq + 1) * 512], trw[:])

        # ---- hstack: rows 0-3 = h, row 4 = W_b, row 5 = w_w ----
        hstack = wt_pool.tile([6, HLEN], F32)
        nc.sync.dma_start(hstack[0:B_LOC, :], h_ext[:, :])
        nc.sync.dma_start(hstack[4:5, :],
                          Wb_ext[:].rearrange("(a v) -> a v", a=1))
        nc.sync.dma_start(hstack[5:6, :],
                          ww_ext[:].rearrange("(a v) -> a v", a=1))

        hT = wt_pool.tile([128, NC_L, B_LOC], BF16)   # h^T chunks (l on part)
        WbT = wt_pool.tile([128, NC_D], F32)          # W_b^T chunks (d on part)
        wwT = wt_pool.tile([128, NC_D], BF16)         # w_w^T chunks (d on part)
        for c in range(NC_L):
            hsT = ps_vp_pool.tile([128, 6], F32, tag="vp")
            if cfg["mm_tr"]:
                nc.tensor.matmul(hsT[:], hstack[:, c * 128:(c + 1) * 128],
                                 id_f32[0:6, 0:6], start=True, stop=True)
            else:
                nc.tensor.transpose(hsT[:], hstack[:, c * 128:(c + 1) * 128],
                                    id_f32[0:6, 0:6])
            nc.vector.tensor_copy(hT[:, c, :], hsT[:, 0:B_LOC])
            nc.vector.tensor_copy(WbT[:, c:c + 1], hsT[:, 4:5])
            nc.vector.tensor_copy(wwT[:, c:c + 1], hsT[:, 5:6])

        # ---- h_projT[d, b] = W_w h + W_b (transposed) ----
        hpT = wt_pool.tile([128, NC_D, B_LOC], F32)
        for dc in range(NC_D):
            ph = ps_vp_pool.tile([128, B_LOC], F32, tag="vp")
            for lc in range(NC_L):
                nc.tensor.matmul(ph[:], WwT[:, lc, dc * 128:(dc + 1) * 128],
                                 hT[:, lc, :],
                                 start=(lc == 0), stop=(lc == NC_L - 1))
            nc.vector.tensor_scalar_add(hpT[:, dc, :], ph[:], WbT[:, dc:dc + 1])

        # ---- main per-batch loop ----
        # (repeat > 1 re-runs the computation inside a dynamic loop for
        #  wall-clock timing probes: T ~ (wall(R2) - wall(R1)) / (R2 - R1))
        loop_cm = tc.For_i(0, repeat, 1) if repeat > 1 else None
        if loop_cm is not None:
            ctx.enter_context(loop_cm)

        def emit_nat_load(nat, b):
            if "dma" in ablate:
                nc.gpsimd.memset(nat[:, 0, 0:8], 0.5)  # keep tile "written"
            elif cfg["batch_dma"]:
                nc.gpsimd.dma_start(
                    nat[:, :, :], V_ext[b].rearrange("(t p) v -> p t v", p=128))
            else:
                for t in range(NT):
                    nc.gpsimd.dma_start(nat[:, t, :],
                                        V_ext[b, t * 128:(t + 1) * 128, :])

        def emit_tr_group(vt, nat, c, q):
            if cfg["xbar_vt"]:
                for j in range(4):
                    t = 4 * q + j
                    nc.sync.dma_start(vt[:, c, t * 128:(t + 1) * 128],
                                      nat[:, t, c * 128:(c + 1) * 128],
                                      transpose=True)
                return
            if cfg["mm_tr"]:
                # transpose as a REGULAR matmul against identity: pipelines
                # like normal matmuls and avoids is_transpose Ldweights
                trf = ps_tr_pool.tile([128, 512], F32, tag="tr")
                for j in range(4):
                    t = 4 * q + j
                    nc.tensor.matmul(trf[:, j * 128:(j + 1) * 128],
                                     nat[:, t, c * 128:(c + 1) * 128],
                                     id_bf[:, :], start=True, stop=True)
                dst = vt[:, c, q * 512:(q + 1) * 512]
                if cfg["alt_copy"] and (c * 4 + q) % 2 == 1:
                    nc.scalar.copy(dst, trf[:])
                else:
                    nc.vector.tensor_copy(dst, trf[:])
                return
            tr = ps_tr_pool.tile([128, 512], BF16, tag="tr")
            for j in range(4):
                t = 4 * q + j
                nc.tensor.transpose(tr[:, j * 128:(j + 1) * 128],
                                    nat[:, t, c * 128:(c + 1) * 128],
                                    id_bf[:, :])
            nc.vector.tensor_copy(vt[:, c, q * 512:(q + 1) * 512], tr[:])

        def emit_vt(vt, nat):
            if "vt" in ablate:
                nc.gpsimd.memset(vt[:, 0, 0:8], 0.5)   # keep tile "written"
            else:
                for c in range(NC_V):
                    for q in range(NT // 4):
                        emit_tr_group(vt, nat, c, q)

        nats = {}
        vts = {}
        for b in range(B_LOC):
            if cfg["pipelined"]:
                # transposes of batch b+1 are interleaved into pass 1 of
                # batch b below; batch 0 does its own upfront
                if b == 0:
                    nats[0] = nat_pool.tile([128, NT, VLEN], BF16, tag="nat",
                                            name="nat0")
                    emit_nat_load(nats[0], 0)
                    vts[0] = vt_pool.tile([128, NC_V, S], BF16, tag="vt",
                                          name="vt0")
                    emit_vt(vts[0], nats[0])
                if b + 1 < B_LOC:
                    nats[b + 1] = nat_pool.tile([128, NT, VLEN], BF16,
                                                tag="nat", name=f"nat{b + 1}")
                    emit_nat_load(nats[b + 1], b + 1)
                    vts[b + 1] = vt_pool.tile([128, NC_V, S], BF16, tag="vt",
                                              name=f"vt{b + 1}")
                nat, vt = nats[b], vts[b]
            else:
                # V natural layout, cast to bf16 in DMA: nat[s_part, t, v]
                nat = nat_pool.tile([128, NT, VLEN], BF16, tag="nat")
                emit_nat_load(nat, b)
                # V^T via PE transpose: vt[v_part, c, s]
                vt = vt_pool.tile([128, NC_V, S], BF16, tag="vt")
                emit_vt(vt, nat)

            # pass 1: e[s] = sum_d w_d tanh(h_proj_d + (U V^T)[d, s]),
            # accumulated as natural rows [1, 512], one PSUM group per s-tile
            zparts = o_pool.tile([1, NST], F32, tag="zp")
            p_nat = p_pool.tile([1, S], BF16, tag="pnat")
            nvc = 1 if "vp" in ablate else NC_V

            def emit_exp(st, er):
                # softmax numerator (e bounded, skip max-subtraction);
                # accum_out produces the partial sum of exp for Z
                nc.scalar.activation(p_nat[0:1, st * 512:(st + 1) * 512],
                                     er[:], AF.Exp,
                                     accum_out=zparts[0:1, st:st + 1])

            if cfg["st_pair"]:
                # two s-tiles per dc: each UwT block is loaded once as the
                # stationary operand and used for two matmuls
                for sp in range(NST // 2):
                    sts = (2 * sp, 2 * sp + 1)
                    ers = {st: ps_e_pool.tile([1, 512], F32, tag="er",
                                              name=f"er{st}")
                           for st in sts}
                    for dc in range(NC_D):
                        vps = {st: ps_vp_pool.tile([128, 512], F32,
                                                    tag="vp", name=f"vp{st}")
                               for st in sts}
                        for vc in range(nvc):
                            for st in sts:
                                nc.tensor.matmul(
                                    vps[st][:],
                                    UwT[:, vc, dc * 128:(dc + 1) * 128],
                                    vt[:, vc, st * 512:(st + 1) * 512],
                                    start=(vc == 0), stop=(vc == nvc - 1))
                        for st in sts:
                            th = tanh_pool.tile([128, 512], BF16, tag="th")
                            nc.scalar.activation(th[:], vps[st][:], AF.Tanh,
                                                 bias=hpT[:, dc, b:b + 1])
                            nc.tensor.matmul(ers[st][:], wwT[:, dc:dc + 1],
                                             th[:], start=(dc == 0),
                                             stop=(dc == NC_D - 1))
                    for st in sts:
                        emit_exp(st, ers[st])
            else:
                for st in range(NST):
                    er = ps_e_pool.tile([1, 512], F32, tag="er")
                    for dc in range(NC_D):
                        vp = ps_vp_pool.tile([128, 512], F32, tag="vp")
                        for vc in range(nvc):
                            nc.tensor.matmul(
                                vp[:], UwT[:, vc, dc * 128:(dc + 1) * 128],
                                vt[:, vc, st * 512:(st + 1) * 512],
                                start=(vc == 0), stop=(vc == nvc - 1))
                        th = tanh_pool.tile([128, 512], BF16, tag="th")
                        nc.scalar.activation(th[:], vp[:], AF.Tanh,
                                             bias=hpT[:, dc, b:b + 1])
                        nc.tensor.matmul(er[:], wwT[:, dc:dc + 1], th[:],
                                         start=(dc == 0),
                                         stop=(dc == NC_D - 1))
                        if (cfg["pipelined"] and b + 1 < B_LOC
                                and "vt" not in ablate):
                            i = st * NC_D + dc
                            emit_tr_group(vts[b + 1], nats[b + 1],
                                          i // 4, i % 4)
                    emit_exp(st, er)

            # transpose p rows back to columns for pass-2 lhsT
            # (pad columns to 4B: PSUM accesses must be 4-byte aligned)
            pt_pool = ps_o_pool if cfg["merge_popt"] else ps_pt_pool
            pt_tag = "po" if cfg["merge_popt"] else "pt"
            if cfg["mm_tr"]:
                ps_ptf = pt_pool.tile([128, NT], F32, tag=pt_tag, name="ps_ptf")
                for t in range(NT):
                    nc.tensor.matmul(ps_ptf[:, t:t + 1],
                                     p_nat[0:1, t * 128:(t + 1) * 128],
                                     id_bf[0:1, 0:1], start=True, stop=True)
                pT = p_pool.tile([128, NT], BF16, tag="pT")
                nc.vector.tensor_copy(pT[:], ps_ptf[:])
            else:
                ps_pt = pt_pool.tile([128, NT, 2], BF16, tag=pt_tag)
                for t in range(NT):
                    nc.tensor.transpose(ps_pt[:, t, 0:1],
                                        p_nat[0:1, t * 128:(t + 1) * 128],
                                        id_bf[0:1, 0:1])
                pT = p_pool.tile([128, NT], BF16, tag="pT")
                nc.vector.tensor_copy(pT[:], ps_pt[:, :, 0])

            # pass 2: out_un[v] = sum_s p_s V[s, v]
            out_un = o_pool.tile([1, VLEN], F32, tag="oun")
            ntp = 1 if "p2" in ablate else NT
            for half in range(2):
                po = ps_o_pool.tile([1, 512], F32, tag="po")
                for t in range(ntp):
                    nc.tensor.matmul(po[:], pT[:, t:t + 1],
                                     nat[:, t, half * 512:(half + 1) * 512],
                                     start=(t == 0), stop=(t == ntp - 1))
                nc.vector.tensor_copy(out_un[0:1, half * 512:(half + 1) * 512],
                                      po[:])
            zsum = o_pool.tile([1, 1], F32, tag="zs")
            nc.vector.reduce_sum(zsum[:], zparts[:], axis=mybir.AxisListType.X)
            zinv = o_pool.tile([1, 1], F32, tag="zi")
            nc.vector.reciprocal(zinv[:], zsum[:])
            out_fin = o_pool.tile([1, VLEN], F32, tag="ofin")
            nc.vector.tensor_scalar_mul(out_fin[:], out_un[:], zinv[:])
            nc.sync.dma_start(out_ext[b:b + 1, :], out_fin[:])

    nc.compile()
    return nc


_cached_nc = None


def _get_nc():
    global _cached_nc
    if _cached_nc is None:
        _cached_nc = build_kernel()
    return _cached_nc


def make_in_maps(h, V, W_w, W_b, U_w, w_w):
    h = np.ascontiguousarray(np.asarray(h, dtype=np.float32))
    V = np.ascontiguousarray(np.asarray(V, dtype=np.float32))
    W_w = np.ascontiguousarray(np.asarray(W_w, dtype=np.float32))
    W_b = np.ascontiguousarray(np.asarray(W_b, dtype=np.float32))
    U_w = np.ascontiguousarray(np.asarray(U_w, dtype=np.float32))
    w_w = np.ascontiguousarray(np.asarray(w_w, dtype=np.float32))
    in_maps = []
    for i in range(N_CORES):
        sl = slice(i * B_LOC, (i + 1) * B_LOC)
        in_maps.append({
            "h": h[sl], "V": V[sl],
            "W_w": W_w, "W_b": W_b, "U_w": U_w, "w_w": w_w,
        })
    return in_maps


def kernel(h, V, W_w, W_b, U_w, w_w):
    nc = _get_nc()
    in_maps = make_in_maps(h, V, W_w, W_b, U_w, w_w)
    res = run_bass_kernel_spmd(nc, in_maps, list(range(N_CORES)))
    out = np.concatenate([res.results[i]["out"] for i in range(N_CORES)],
                         axis=0)
    return np.asarray(out, dtype=np.float32)


if __name__ == "__main__":
    rng = np.random.default_rng(0)
    h = rng.standard_normal((B, HLEN), dtype=np.float32)
    V = rng.standard_normal((B, S, VLEN), dtype=np.float32)
    W_w = rng.standard_normal((HID, HLEN), dtype=np.float32) / np.sqrt(HLEN)
    W_b = rng.standard_normal((HID,), dtype=np.float32) / np.sqrt(HLEN)
    U_w = rng.standard_normal((HID, VLEN), dtype=np.float32) / np.sqrt(VLEN)
    w_w = rng.standard_normal((HID,), dtype=np.float32) / np.sqrt(HID)
    out = kernel(h, V, W_w, W_b, U_w, w_w)
    print("out", out.shape, out.dtype, float(np.abs(out).mean()))
